# revision 7
# baseline (speedup 1.0000x reference)
"""Trainium2 Bass kernel for EnhancedAttention (B=2, T=2048, D=1024, H=16, DH=64).

Sharding: 8 cores = 2 batches x 4 head-groups (4 heads each). No collectives;
each core computes a partial out-projection and the host sums the 4 partials
per batch.

Per-core dataflow (all "transposed" space, float32r matmuls):
  Q^T,K^T [dh,t] from W^T @ x^T; RoPE via strip-swap DMA + full-width DVE ops;
  V natural [t,dh] augmented with a ones column so softmax denominators fall
  out of the attention matmul; causal handled by skipping above-diagonal
  k-tiles entirely + gpsimd.affine_select on diagonal tiles; normalization
  applied to O^T (per-head) before the out-projection.
"""
import os
import sys

for _p in ("/opt/trn_rl_repo", "/root/.axon_site/_ro/trn_rl_repo"):
    if os.path.isdir(_p) and _p not in sys.path:
        sys.path.append(_p)

import numpy as np

import concourse.bass as bass  # noqa: F401
import concourse.tile as tile
from concourse import bacc, mybir
from concourse.bass_utils import run_bass_kernel_spmd

B, T, D = 2, 2048, 1024
H, DH = 16, 64
HPC = 4  # heads per core
NCORES = 8
ROPE_THETA = 10000.0

F32 = mybir.dt.float32
F32R = mybir.dt.float32r

TCH = 512  # t-chunk (q-chunk) size
TC = T // TCH  # 4
DC = D // 128  # 8 contraction chunks
NKT = T // 128  # 16 k-tiles


def _rope_tables():
    inv = 1.0 / (ROPE_THETA ** (np.arange(0, DH, 2, dtype=np.float64) / DH))
    f = np.arange(T, dtype=np.float64)[:, None] * inv[None, :]  # [T, 32]
    cos = np.cos(f).T.astype(np.float32)  # [32, T]
    sin = np.sin(f).T.astype(np.float32)
    cs1 = np.ascontiguousarray(np.tile(cos, (4, 1)))  # [128, T]
    cs2 = np.ascontiguousarray(np.concatenate([-sin, sin, -sin, sin], axis=0))
    return cs1, cs2


def _build():
    nc = bacc.Bacc("TRN2", target_bir_lowering=False, debug=False, num_devices=NCORES)
    xT_d = nc.dram_tensor("xT", [D, T], F32R, kind="ExternalInput")
    wq_d = nc.dram_tensor("wq", [D, HPC * DH], F32R, kind="ExternalInput")
    wk_d = nc.dram_tensor("wk", [D, HPC * DH], F32R, kind="ExternalInput")
    wv_d = nc.dram_tensor("wv", [D, HPC * DH], F32R, kind="ExternalInput")
    wo_d = nc.dram_tensor("wo", [HPC * DH, D], F32R, kind="ExternalInput")
    y_d = nc.dram_tensor("y", [T, D], F32, kind="ExternalOutput")

    cs1_np, cs2_np = _rope_tables()
    cs1_d = nc.inline_tensor(cs1_np, "cs1")
    cs2_d = nc.inline_tensor(cs2_np, "cs2")

    EXP = mybir.ActivationFunctionType.Exp

    with tile.TileContext(nc) as tc:
        with (
            tc.tile_pool(name="sb", bufs=1) as sb,
            tc.tile_pool(name="xtp", bufs=2) as xtp,
            tc.tile_pool(name="ropep", bufs=2) as ropep,
            tc.tile_pool(name="ptp", bufs=4) as ptp,
            tc.tile_pool(name="misp", bufs=2) as misp,
            tc.tile_pool(name="ysbp", bufs=2) as ysbp,
        ):
            wq = sb.tile([128, DC, HPC * DH], F32R)
            wk = sb.tile([128, DC, HPC * DH], F32R)
            wv = sb.tile([128, DC, HPC * DH], F32R)
            wo = sb.tile([128, 2, D], F32R)
            cs1 = sb.tile([128, T], F32)
            cs2 = sb.tile([128, T], F32)
            qt = [sb.tile([128, T], F32R, tag=f"qt{p}", name=f"qt{p}") for p in range(2)]
            ktt = [sb.tile([128, T], F32R, tag=f"kt{p}", name=f"kt{p}") for p in range(2)]
            vaug = sb.tile([128, NKT, HPC, DH + 1], F32R)
            ot = [sb.tile([128, T], F32R, tag=f"ot{p}", name=f"ot{p}") for p in range(2)]

            wq_r = wq_d.ap().rearrange("(c p) n -> p c n", p=128)
            wk_r = wk_d.ap().rearrange("(c p) n -> p c n", p=128)
            wv_r = wv_d.ap().rearrange("(c p) n -> p c n", p=128)
            for dc in range(DC):
                nc.scalar.dma_start(wq[:, dc, :], wq_r[:, dc, :])
                nc.scalar.dma_start(wk[:, dc, :], wk_r[:, dc, :])
                nc.scalar.dma_start(wv[:, dc, :], wv_r[:, dc, :])
            nc.gpsimd.dma_start(wo[:], wo_d.ap().rearrange("(c p) n -> p c n", p=128))
            nc.gpsimd.dma_start(cs1[:], cs1_d.ap())
            nc.gpsimd.dma_start(cs2[:], cs2_d.ap())
            nc.vector.memset(vaug[:, :, :, DH : DH + 1].bitcast(F32), 1.0)

            xT_r = xT_d.ap().rearrange("(c p) t -> p c t", p=128)

            # ---- projection + RoPE, per t-chunk of 512 ----
            with tc.tile_pool(name="pjps", bufs=2, space="PSUM") as pjps:
                # PE warm-up: fills the input-DMA wait so HAM reaches K=8/8
                warm = sb.tile([128, TCH], F32R, name="warm")
                nc.vector.memset(warm.bitcast(F32), 0.0)
                wps = pjps.tile([128, TCH], F32, tag="pj", name="wps")
                for wi in range(16):
                    nc.tensor.matmul(
                        wps[:], warm[:, 0:128], warm[:],
                        start=(wi == 0), stop=(wi == 15),
                    )
                for tci in range(TC):
                    tsl = slice(tci * TCH, (tci + 1) * TCH)
                    xt = xtp.tile([128, DC, TCH], F32R, tag="xt")
                    for dc in range(DC):
                        nc.sync.dma_start(xt[:, dc, :], xT_r[:, dc, tsl])

                    for w_sb, dest in ((wq, qt), (wk, ktt)):
                        for p in range(2):
                            ps = pjps.tile([128, TCH], F32, tag="pj")
                            for dc in range(DC):
                                nc.tensor.matmul(
                                    ps[:],
                                    w_sb[:, dc, p * 128 : (p + 1) * 128],
                                    xt[:, dc, :],
                                    start=(dc == 0),
                                    stop=(dc == DC - 1),
                                )
                            e = ropep.tile([128, TCH], F32, tag="e")
                            sw = ropep.tile([128, TCH], F32, tag="sw")
                            nc.vector.tensor_copy(e[:], ps[:])
                            for s in range(4):
                                nc.scalar.dma_start(
                                    sw[s * 32 : (s + 1) * 32, :],
                                    e[(s ^ 1) * 32 : ((s ^ 1) + 1) * 32, :],
                                )
                            nc.vector.tensor_mul(e[:], e[:], cs1[:, tsl])
                            nc.vector.tensor_mul(sw[:], sw[:], cs2[:, tsl])
                            nc.vector.tensor_add(dest[p][:, tsl], e[:], sw[:])

                    for tt in range(4):
                        gt = tci * 4 + tt  # global t-tile / k-tile index
                        ps = pjps.tile([128, TCH], F32, tag="pj")
                        for dc in range(DC):
                            nc.tensor.matmul(
                                ps[:, : HPC * DH],
                                xt[:, dc, tt * 128 : (tt + 1) * 128],
                                wv[:, dc, :],
                                start=(dc == 0),
                                stop=(dc == DC - 1),
                            )
                        nc.vector.tensor_copy(
                            vaug[:, gt, :, 0:DH],
                            ps[:, : HPC * DH].rearrange("p (h d) -> p h d", h=HPC),
                        )

            # ---- attention + out-projection, per q-chunk ----
            attn_stack = __import__("contextlib").ExitStack()
            sps = attn_stack.enter_context(tc.tile_pool(name="sps", bufs=3, space="PSUM"))
            ops = attn_stack.enter_context(tc.tile_pool(name="ops", bufs=1, space="PSUM"))
            yps = attn_stack.enter_context(tc.tile_pool(name="yps", bufs=1, space="PSUM"))
            for qc in range(TC):
                qsl = slice(qc * TCH, (qc + 1) * TCH)
                nkt = 4 * qc + 4  # causal: k-tiles 0..4qc+3
                for h in range(HPC):
                    pr, par = divmod(h, 2)
                    qrh = qt[pr][par * 64 : par * 64 + 64, qsl]
                    opsum = ops.tile([128, TCH], F32, tag="o")
                    for kb in range(0, nkt, 2):
                        spt = sps.tile([128, 2, TCH], F32, tag="s")
                        pt = ptp.tile([128, 2, TCH], F32R, tag="pt")
                        for j in range(2):
                            ktile = kb + j
                            nc.tensor.matmul(
                                spt[:, j, :],
                                ktt[pr][
                                    par * 64 : par * 64 + 64,
                                    ktile * 128 : (ktile + 1) * 128,
                                ],
                                qrh,
                                start=True,
                                stop=True,
                            )
                        nc.scalar.activation(
                            pt.rearrange("p a b -> p (a b)"),
                            spt.rearrange("p a b -> p (a b)"),
                            EXP,
                            bias=0.0,
                            scale=0.125,
                        )
                        for j in range(2):
                            ktile = kb + j
                            if ktile >= 4 * qc:  # diagonal region: mask k > q
                                nc.gpsimd.affine_select(
                                    out=pt[:, j, :],
                                    in_=pt[:, j, :],
                                    compare_op=mybir.AluOpType.is_ge,
                                    fill=0.0,
                                    base=512 * qc - 128 * ktile,
                                    pattern=[[1, TCH]],
                                    channel_multiplier=-1,
                                )
                        for j in range(2):
                            ktile = kb + j
                            nc.tensor.matmul(
                                opsum[0 : DH + 1, :],
                                vaug[:, ktile, h, :],
                                pt[:, j, :],
                                start=(ktile == 0),
                                stop=(ktile == nkt - 1),
                            )
                    # evac O' fast (frees the PSUM bank), normalize from SBUF
                    oraw = misp.tile([128, TCH], F32, tag="oraw")
                    nc.vector.tensor_copy(oraw[0 : DH + 1, :], opsum[0 : DH + 1, :])
                    rec0 = misp.tile([1, TCH], F32, tag="rec0")
                    nc.gpsimd.dma_start(rec0[:], oraw[DH : DH + 1, :])
                    nc.vector.reciprocal(rec0[:], rec0[:])
                    bc = misp.tile([64, TCH], F32, tag="bc")
                    nc.gpsimd.partition_broadcast(bc[:], rec0[:])
                    if par == 0:
                        nc.vector.tensor_mul(
                            ot[pr][0:64, qsl], oraw[0:64, :], bc[:]
                        )
                    else:
                        tmpo = misp.tile([64, TCH], F32R, tag="tmpo")
                        nc.vector.tensor_mul(tmpo[:], oraw[0:64, :], bc[:])
                        nc.scalar.dma_start(ot[pr][64:128, qsl], tmpo[:])

                for tt in range(4):
                    gtt = qc * 4 + tt
                    for ni in range(2):
                        ypsum = yps.tile([128, TCH], F32, tag="y")
                        for p2 in range(2):
                            nc.tensor.matmul(
                                ypsum[:],
                                ot[p2][:, gtt * 128 : (gtt + 1) * 128],
                                wo[:, p2, ni * TCH : (ni + 1) * TCH],
                                start=(p2 == 0),
                                stop=(p2 == 1),
                            )
                        ysb = ysbp.tile([128, TCH], F32, tag="ysb")
                        nc.vector.tensor_copy(ysb[:], ypsum[:])
                        nc.sync.dma_start(
                            y_d.ap()[
                                gtt * 128 : (gtt + 1) * 128,
                                ni * TCH : (ni + 1) * TCH,
                            ],
                            ysb[:],
                        )
            attn_stack.close()
    nc.compile()
    return nc


_NC_CACHE = []


def _get_nc():
    if not _NC_CACHE:
        _NC_CACHE.append(_build())
    return _NC_CACHE[0]


_LAST_RESULTS = []  # stashed BassKernelResults for test harness introspection


def kernel(x, Wqkv, Wout, _trace=False, **_trace_kwargs):
    x = np.asarray(x, dtype=np.float32)
    Wqkv = np.asarray(Wqkv, dtype=np.float32)
    Wout = np.asarray(Wout, dtype=np.float32)

    nc = _get_nc()
    in_maps = []
    for c in range(NCORES):
        b, g = divmod(c, HPC)
        cols = slice(g * HPC * DH, (g + 1) * HPC * DH)
        rows = slice(g * HPC * DH, (g + 1) * HPC * DH)
        in_maps.append(
            {
                "xT": np.ascontiguousarray(x[b].T),
                "wq": np.ascontiguousarray(Wqkv[:, 0:D][:, cols]),
                "wk": np.ascontiguousarray(Wqkv[:, D : 2 * D][:, cols]),
                "wv": np.ascontiguousarray(Wqkv[:, 2 * D : 3 * D][:, cols]),
                "wo": np.ascontiguousarray(Wout[rows, :]),
                "y": None,  # outputs are allocated by the runner
            }
        )
    for m in in_maps:
        m.pop("y")

    res = run_bass_kernel_spmd(
        nc, in_maps, core_ids=list(range(NCORES)), trace=_trace, **_trace_kwargs
    )
    _LAST_RESULTS.clear()
    _LAST_RESULTS.append(res)

    out = np.zeros((B, T, D), dtype=np.float32)
    for c in range(NCORES):
        b = c // HPC
        out[b] += res.results[c]["y"]
    return out


# revision 8
# speedup vs baseline: 1.0064x; 1.0064x over previous
"""Trainium2 Bass kernel for EnhancedAttention (B=2, T=2048, D=1024, H=16, DH=64).

Sharding: 8 cores = 2 batches x 4 head-groups (4 heads each). No collectives;
each core computes a partial out-projection and the host sums the 4 partials
per batch.

Per-core dataflow (all "transposed" space, float32r matmuls):
  Q^T,K^T [dh,t] from W^T @ x^T; RoPE via strip-swap DMA + full-width DVE ops;
  V natural [t,dh] augmented with a ones column so softmax denominators fall
  out of the attention matmul; causal handled by skipping above-diagonal
  k-tiles entirely + gpsimd.affine_select on diagonal tiles; normalization
  applied to O^T (per-head) before the out-projection.
"""
import os
import sys

for _p in ("/opt/trn_rl_repo", "/root/.axon_site/_ro/trn_rl_repo"):
    if os.path.isdir(_p) and _p not in sys.path:
        sys.path.append(_p)

import numpy as np

import concourse.bass as bass  # noqa: F401
import concourse.tile as tile
from concourse import bacc, mybir
from concourse.bass_utils import run_bass_kernel_spmd

B, T, D = 2, 2048, 1024
H, DH = 16, 64
HPC = 4  # heads per core
NCORES = 8
ROPE_THETA = 10000.0

F32 = mybir.dt.float32
F32R = mybir.dt.float32r

TCH = 512  # t-chunk (q-chunk) size
TC = T // TCH  # 4
DC = D // 128  # 8 contraction chunks
NKT = T // 128  # 16 k-tiles


def _rope_tables():
    inv = 1.0 / (ROPE_THETA ** (np.arange(0, DH, 2, dtype=np.float64) / DH))
    f = np.arange(T, dtype=np.float64)[:, None] * inv[None, :]  # [T, 32]
    cos = np.cos(f).T.astype(np.float32)  # [32, T]
    sin = np.sin(f).T.astype(np.float32)
    cs1 = np.ascontiguousarray(np.tile(cos, (4, 1)))  # [128, T]
    cs2 = np.ascontiguousarray(np.concatenate([-sin, sin, -sin, sin], axis=0))
    return cs1, cs2


def _build():
    nc = bacc.Bacc("TRN2", target_bir_lowering=False, debug=False, num_devices=NCORES)
    xT_d = nc.dram_tensor("xT", [D, T], F32R, kind="ExternalInput")
    wq_d = nc.dram_tensor("wq", [D, HPC * DH], F32R, kind="ExternalInput")
    wk_d = nc.dram_tensor("wk", [D, HPC * DH], F32R, kind="ExternalInput")
    wv_d = nc.dram_tensor("wv", [D, HPC * DH], F32R, kind="ExternalInput")
    wo_d = nc.dram_tensor("wo", [HPC * DH, D], F32R, kind="ExternalInput")
    y_d = nc.dram_tensor("y", [T, D], F32, kind="ExternalOutput")

    cs1_np, cs2_np = _rope_tables()
    cs1_d = nc.inline_tensor(cs1_np, "cs1")
    cs2_d = nc.inline_tensor(cs2_np, "cs2")

    EXP = mybir.ActivationFunctionType.Exp

    import contextlib
    with tile.TileContext(nc) as tc:
        with (
            contextlib.ExitStack() as _ctx,
            tc.tile_pool(name="sb", bufs=1) as sb,
            tc.tile_pool(name="xtp", bufs=2) as xtp,
            tc.tile_pool(name="ropep", bufs=2) as ropep,
            tc.tile_pool(name="ptp", bufs=4) as ptp,
            tc.tile_pool(name="misp", bufs=2) as misp,
            tc.tile_pool(name="ysbp", bufs=2) as ysbp,
        ):
            wq = sb.tile([128, DC, HPC * DH], F32R)
            wk = sb.tile([128, DC, HPC * DH], F32R)
            wv = sb.tile([128, DC, HPC * DH], F32R)
            wo = sb.tile([128, 2, D], F32R)
            cs1 = sb.tile([128, T], F32)
            cs2 = sb.tile([128, T], F32)
            qt = [sb.tile([128, T], F32R, tag=f"qt{p}", name=f"qt{p}") for p in range(2)]
            ktt = [sb.tile([128, T], F32R, tag=f"kt{p}", name=f"kt{p}") for p in range(2)]
            vaug = sb.tile([128, NKT, HPC, DH + 1], F32R)
            ot = [sb.tile([128, T], F32R, tag=f"ot{p}", name=f"ot{p}") for p in range(2)]

            wq_r = wq_d.ap().rearrange("(c p) n -> p c n", p=128)
            wk_r = wk_d.ap().rearrange("(c p) n -> p c n", p=128)
            wv_r = wv_d.ap().rearrange("(c p) n -> p c n", p=128)
            for dc in range(DC):
                nc.scalar.dma_start(wq[:, dc, :], wq_r[:, dc, :])
                nc.scalar.dma_start(wk[:, dc, :], wk_r[:, dc, :])
                nc.scalar.dma_start(wv[:, dc, :], wv_r[:, dc, :])
            nc.gpsimd.dma_start(wo[:], wo_d.ap().rearrange("(c p) n -> p c n", p=128))
            nc.gpsimd.dma_start(cs1[:], cs1_d.ap())
            nc.gpsimd.dma_start(cs2[:], cs2_d.ap())
            nc.vector.memset(vaug[:, :, :, DH : DH + 1].bitcast(F32), 1.0)

            xT_r = xT_d.ap().rearrange("(c p) t -> p c t", p=128)

            # One flat PSUM pool set so proj and attention interleave freely
            pjps = _ctx.enter_context(tc.tile_pool(name="pjps", bufs=2, space="PSUM"))
            sps = _ctx.enter_context(tc.tile_pool(name="sps", bufs=2, space="PSUM"))
            ops = _ctx.enter_context(tc.tile_pool(name="ops", bufs=1, space="PSUM"))
            yps = _ctx.enter_context(tc.tile_pool(name="yps", bufs=1, space="PSUM"))

            # PE warm-up: fills the input-DMA wait so HAM reaches K=8/8
            warm = sb.tile([128, TCH], F32R, name="warm")
            nc.vector.memset(warm.bitcast(F32), 0.0)
            wps = pjps.tile([128, TCH], F32, tag="pj", name="wps")
            for wi in range(16):
                nc.tensor.matmul(
                    wps[:], warm[:, 0:128], warm[:],
                    start=(wi == 0), stop=(wi == 15),
                )

            def proj_chunk(tci):
                tsl = slice(tci * TCH, (tci + 1) * TCH)
                xt = xtp.tile([128, DC, TCH], F32R, tag="xt", name=f"xt{tci}")
                for dc in range(DC):
                    nc.sync.dma_start(xt[:, dc, :], xT_r[:, dc, tsl])

                for w_sb, dest in ((wq, qt), (wk, ktt)):
                    for p in range(2):
                        ps = pjps.tile([128, TCH], F32, tag="pj", name=f"pj{tci}_{p}")
                        for dc in range(DC):
                            nc.tensor.matmul(
                                ps[:],
                                w_sb[:, dc, p * 128 : (p + 1) * 128],
                                xt[:, dc, :],
                                start=(dc == 0),
                                stop=(dc == DC - 1),
                            )
                        e = ropep.tile([128, TCH], F32, tag="e", name=f"e{tci}")
                        sw = ropep.tile([128, TCH], F32, tag="sw", name=f"sw{tci}")
                        nc.vector.tensor_copy(e[:], ps[:])
                        for s in range(4):
                            nc.scalar.dma_start(
                                sw[s * 32 : (s + 1) * 32, :],
                                e[(s ^ 1) * 32 : ((s ^ 1) + 1) * 32, :],
                            )
                        nc.vector.tensor_mul(e[:], e[:], cs1[:, tsl])
                        nc.vector.tensor_mul(sw[:], sw[:], cs2[:, tsl])
                        nc.vector.tensor_add(dest[p][:, tsl], e[:], sw[:])

                for tt in range(4):
                    gt = tci * 4 + tt  # global t-tile / k-tile index
                    ps = pjps.tile([128, TCH], F32, tag="pj", name=f"pjv{gt}")
                    for dc in range(DC):
                        nc.tensor.matmul(
                            ps[:, : HPC * DH],
                            xt[:, dc, tt * 128 : (tt + 1) * 128],
                            wv[:, dc, :],
                            start=(dc == 0),
                            stop=(dc == DC - 1),
                        )
                    nc.vector.tensor_copy(
                        vaug[:, gt, :, 0:DH],
                        ps[:, : HPC * DH].rearrange("p (h d) -> p h d", h=HPC),
                    )

            def attn_chunk(qc):
                qsl = slice(qc * TCH, (qc + 1) * TCH)
                nkt = 4 * qc + 4  # causal: k-tiles 0..4qc+3
                for h in range(HPC):
                    pr, par = divmod(h, 2)
                    qrh = qt[pr][par * 64 : par * 64 + 64, qsl]
                    opsum = ops.tile([128, TCH], F32, tag="o", name=f"o{qc}_{h}")
                    for kb in range(0, nkt, 2):
                        spt = sps.tile([128, 2, TCH], F32, tag="s", name=f"s{qc}_{h}")
                        pt = ptp.tile([128, 2, TCH], F32R, tag="pt", name=f"pt{qc}_{h}")
                        for j in range(2):
                            ktile = kb + j
                            nc.tensor.matmul(
                                spt[:, j, :],
                                ktt[pr][
                                    par * 64 : par * 64 + 64,
                                    ktile * 128 : (ktile + 1) * 128,
                                ],
                                qrh,
                                start=True,
                                stop=True,
                            )
                        nc.scalar.activation(
                            pt.rearrange("p a b -> p (a b)"),
                            spt.rearrange("p a b -> p (a b)"),
                            EXP,
                            bias=0.0,
                            scale=0.125,
                        )
                        for j in range(2):
                            ktile = kb + j
                            if ktile >= 4 * qc:  # diagonal region: mask k > q
                                nc.gpsimd.affine_select(
                                    out=pt[:, j, :],
                                    in_=pt[:, j, :],
                                    compare_op=mybir.AluOpType.is_ge,
                                    fill=0.0,
                                    base=512 * qc - 128 * ktile,
                                    pattern=[[1, TCH]],
                                    channel_multiplier=-1,
                                )
                        for j in range(2):
                            ktile = kb + j
                            nc.tensor.matmul(
                                opsum[0 : DH + 1, :],
                                vaug[:, ktile, h, :],
                                pt[:, j, :],
                                start=(ktile == 0),
                                stop=(ktile == nkt - 1),
                            )
                    # evac O' fast (frees the PSUM bank), normalize from SBUF
                    oraw = misp.tile([128, TCH], F32, tag="oraw", name=f"or{qc}_{h}")
                    nc.vector.tensor_copy(oraw[0 : DH + 1, :], opsum[0 : DH + 1, :])
                    rec0 = misp.tile([1, TCH], F32, tag="rec0", name=f"rc{qc}_{h}")
                    nc.gpsimd.dma_start(rec0[:], oraw[DH : DH + 1, :])
                    nc.vector.reciprocal(rec0[:], rec0[:])
                    bc = misp.tile([64, TCH], F32, tag="bc", name=f"bc{qc}_{h}")
                    nc.gpsimd.partition_broadcast(bc[:], rec0[:])
                    if par == 0:
                        nc.vector.tensor_mul(
                            ot[pr][0:64, qsl], oraw[0:64, :], bc[:]
                        )
                    else:
                        tmpo = misp.tile([64, TCH], F32R, tag="tmpo", name=f"tp{qc}_{h}")
                        nc.vector.tensor_mul(tmpo[:], oraw[0:64, :], bc[:])
                        nc.scalar.dma_start(ot[pr][64:128, qsl], tmpo[:])

                for tt in range(4):
                    gtt = qc * 4 + tt
                    for ni in range(2):
                        ypsum = yps.tile([128, TCH], F32, tag="y", name=f"y{gtt}_{ni}")
                        for p2 in range(2):
                            nc.tensor.matmul(
                                ypsum[:],
                                ot[p2][:, gtt * 128 : (gtt + 1) * 128],
                                wo[:, p2, ni * TCH : (ni + 1) * TCH],
                                start=(p2 == 0),
                                stop=(p2 == 1),
                            )
                        ysb = ysbp.tile([128, TCH], F32, tag="ysb", name=f"ys{gtt}_{ni}")
                        nc.vector.tensor_copy(ysb[:], ypsum[:])
                        nc.sync.dma_start(
                            y_d.ap()[
                                gtt * 128 : (gtt + 1) * 128,
                                ni * TCH : (ni + 1) * TCH,
                            ],
                            ysb[:],
                        )

            # Interleaved emission: proj stays one chunk ahead of attention so
            # PE always has dense independent work during softmax chain stalls.
            proj_chunk(0)
            proj_chunk(1)
            attn_chunk(0)
            proj_chunk(2)
            attn_chunk(1)
            proj_chunk(3)
            attn_chunk(2)
            attn_chunk(3)
    nc.compile()
    return nc


_NC_CACHE = []


def _get_nc():
    if not _NC_CACHE:
        _NC_CACHE.append(_build())
    return _NC_CACHE[0]


_LAST_RESULTS = []  # stashed BassKernelResults for test harness introspection


def kernel(x, Wqkv, Wout, _trace=False, **_trace_kwargs):
    x = np.asarray(x, dtype=np.float32)
    Wqkv = np.asarray(Wqkv, dtype=np.float32)
    Wout = np.asarray(Wout, dtype=np.float32)

    nc = _get_nc()
    in_maps = []
    for c in range(NCORES):
        b, g = divmod(c, HPC)
        cols = slice(g * HPC * DH, (g + 1) * HPC * DH)
        rows = slice(g * HPC * DH, (g + 1) * HPC * DH)
        in_maps.append(
            {
                "xT": np.ascontiguousarray(x[b].T),
                "wq": np.ascontiguousarray(Wqkv[:, 0:D][:, cols]),
                "wk": np.ascontiguousarray(Wqkv[:, D : 2 * D][:, cols]),
                "wv": np.ascontiguousarray(Wqkv[:, 2 * D : 3 * D][:, cols]),
                "wo": np.ascontiguousarray(Wout[rows, :]),
                "y": None,  # outputs are allocated by the runner
            }
        )
    for m in in_maps:
        m.pop("y")

    res = run_bass_kernel_spmd(
        nc, in_maps, core_ids=list(range(NCORES)), trace=_trace, **_trace_kwargs
    )
    _LAST_RESULTS.clear()
    _LAST_RESULTS.append(res)

    out = np.zeros((B, T, D), dtype=np.float32)
    for c in range(NCORES):
        b = c // HPC
        out[b] += res.results[c]["y"]
    return out


# revision 10
# speedup vs baseline: 1.0286x; 1.0221x over previous
"""Trainium2 Bass kernel for EnhancedAttention (B=2, T=2048, D=1024, H=16, DH=64).

Sharding: 8 cores = 2 batches x 4 head-groups (4 heads each). No collectives;
each core computes a partial out-projection and the host sums the 4 partials
per batch.

Per-core dataflow (all "transposed" space, float32r matmuls):
  Q^T,K^T [dh,t] from W^T @ x^T; RoPE via strip-swap DMA + full-width DVE ops;
  V natural [t,dh] augmented with a ones column so softmax denominators fall
  out of the attention matmul; causal handled by skipping above-diagonal
  k-tiles entirely + gpsimd.affine_select on diagonal tiles; normalization
  applied to O^T (per-head) before the out-projection.
"""
import os
import sys

for _p in ("/opt/trn_rl_repo", "/root/.axon_site/_ro/trn_rl_repo"):
    if os.path.isdir(_p) and _p not in sys.path:
        sys.path.append(_p)

import numpy as np

import concourse.bass as bass  # noqa: F401
import concourse.tile as tile
from concourse import bacc, mybir
from concourse.bass_utils import run_bass_kernel_spmd

B, T, D = 2, 2048, 1024
H, DH = 16, 64
HPC = 4  # heads per core
NCORES = 8
ROPE_THETA = 10000.0

F32 = mybir.dt.float32
F32R = mybir.dt.float32r

TCH = 512  # t-chunk (q-chunk) size
TC = T // TCH  # 4
DC = D // 128  # 8 contraction chunks
NKT = T // 128  # 16 k-tiles


def _rope_tables():
    inv = 1.0 / (ROPE_THETA ** (np.arange(0, DH, 2, dtype=np.float64) / DH))
    f = np.arange(T, dtype=np.float64)[:, None] * inv[None, :]  # [T, 32]
    cos = np.cos(f).T.astype(np.float32)  # [32, T]
    sin = np.sin(f).T.astype(np.float32)
    cs1 = np.ascontiguousarray(np.tile(cos, (4, 1)))  # [128, T]
    cs2 = np.ascontiguousarray(np.concatenate([-sin, sin, -sin, sin], axis=0))
    return cs1, cs2


def _build():
    nc = bacc.Bacc("TRN2", target_bir_lowering=False, debug=False, num_devices=NCORES)
    xT_d = nc.dram_tensor("xT", [D, T], F32R, kind="ExternalInput")
    wq_d = nc.dram_tensor("wq", [D, HPC * DH], F32R, kind="ExternalInput")
    wk_d = nc.dram_tensor("wk", [D, HPC * DH], F32R, kind="ExternalInput")
    wv_d = nc.dram_tensor("wv", [D, HPC * DH], F32R, kind="ExternalInput")
    wo_d = nc.dram_tensor("wo", [HPC * DH, D], F32R, kind="ExternalInput")
    y_d = nc.dram_tensor("y", [T, D], F32, kind="ExternalOutput")

    cs1_np, cs2_np = _rope_tables()
    cs1_d = nc.inline_tensor(cs1_np, "cs1")
    cs2_d = nc.inline_tensor(cs2_np, "cs2")

    EXP = mybir.ActivationFunctionType.Exp

    import contextlib
    with tile.TileContext(nc) as tc:
        with (
            contextlib.ExitStack() as _ctx,
            tc.tile_pool(name="sb", bufs=1) as sb,
            tc.tile_pool(name="xtp", bufs=2) as xtp,
            tc.tile_pool(name="ropep", bufs=2) as ropep,
            tc.tile_pool(name="ptp", bufs=6) as ptp,
            tc.tile_pool(name="misp", bufs=2) as misp,
            tc.tile_pool(name="ysbp", bufs=2) as ysbp,
        ):
            wq = sb.tile([128, DC, HPC * DH], F32R)
            wk = sb.tile([128, DC, HPC * DH], F32R)
            wv = sb.tile([128, DC, HPC * DH], F32R)
            wo = sb.tile([128, 2, D], F32R)
            cs1 = sb.tile([128, T], F32)
            cs2 = sb.tile([128, T], F32)
            qt = [sb.tile([128, T], F32R, tag=f"qt{p}", name=f"qt{p}") for p in range(2)]
            ktt = [sb.tile([128, T], F32R, tag=f"kt{p}", name=f"kt{p}") for p in range(2)]
            vaug = sb.tile([128, NKT, HPC, DH + 1], F32R)
            ot = [sb.tile([128, T], F32R, tag=f"ot{p}", name=f"ot{p}") for p in range(2)]

            wq_r = wq_d.ap().rearrange("(c p) n -> p c n", p=128)
            wk_r = wk_d.ap().rearrange("(c p) n -> p c n", p=128)
            wv_r = wv_d.ap().rearrange("(c p) n -> p c n", p=128)
            for dc in range(DC):
                nc.scalar.dma_start(wq[:, dc, :], wq_r[:, dc, :])
                nc.scalar.dma_start(wk[:, dc, :], wk_r[:, dc, :])
                nc.scalar.dma_start(wv[:, dc, :], wv_r[:, dc, :])
            nc.gpsimd.dma_start(wo[:], wo_d.ap().rearrange("(c p) n -> p c n", p=128))
            nc.gpsimd.dma_start(cs1[:], cs1_d.ap())
            nc.gpsimd.dma_start(cs2[:], cs2_d.ap())
            nc.vector.memset(vaug[:, :, :, DH : DH + 1].bitcast(F32), 1.0)

            xT_r = xT_d.ap().rearrange("(c p) t -> p c t", p=128)

            # One flat PSUM pool set so proj and attention interleave freely
            pjps = _ctx.enter_context(tc.tile_pool(name="pjps", bufs=1, space="PSUM"))
            sps = _ctx.enter_context(tc.tile_pool(name="sps", bufs=2, space="PSUM"))
            ops = _ctx.enter_context(tc.tile_pool(name="ops", bufs=2, space="PSUM"))
            yps = _ctx.enter_context(tc.tile_pool(name="yps", bufs=1, space="PSUM"))

            # PE warm-up: fills the input-DMA wait so HAM reaches K=8/8
            warm = sb.tile([128, TCH], F32R, name="warm")
            nc.vector.memset(warm.bitcast(F32), 0.0)
            wps = pjps.tile([128, TCH], F32, tag="pj", name="wps")
            for wi in range(16):
                nc.tensor.matmul(
                    wps[:], warm[:, 0:128], warm[:],
                    start=(wi == 0), stop=(wi == 15),
                )

            def proj_chunk(tci):
                tsl = slice(tci * TCH, (tci + 1) * TCH)
                xt = xtp.tile([128, DC, TCH], F32R, tag="xt", name=f"xt{tci}")
                nc.sync.dma_start(xt[:, 0:4, :], xT_r[:, 0:4, tsl])
                nc.sync.dma_start(xt[:, 4:8, :], xT_r[:, 4:8, tsl])

                for w_sb, dest in ((wq, qt), (wk, ktt)):
                    for p in range(2):
                        ps = pjps.tile([128, TCH], F32, tag="pj", name=f"pj{tci}_{p}")
                        for dc in range(DC):
                            nc.tensor.matmul(
                                ps[:],
                                w_sb[:, dc, p * 128 : (p + 1) * 128],
                                xt[:, dc, :],
                                start=(dc == 0),
                                stop=(dc == DC - 1),
                            )
                        e = ropep.tile([128, TCH], F32, tag="e", name=f"e{tci}")
                        sw = ropep.tile([128, TCH], F32, tag="sw", name=f"sw{tci}")
                        nc.vector.tensor_copy(e[:], ps[:])
                        for s in range(4):
                            ring = (nc.sync, nc.scalar, nc.gpsimd)[(2 * tci + p + s) % 3]
                            ring.dma_start(
                                sw[s * 32 : (s + 1) * 32, :],
                                e[(s ^ 1) * 32 : ((s ^ 1) + 1) * 32, :],
                            )
                        nc.vector.tensor_mul(e[:], e[:], cs1[:, tsl])
                        nc.vector.tensor_mul(sw[:], sw[:], cs2[:, tsl])
                        nc.vector.tensor_add(dest[p][:, tsl], e[:], sw[:])

                for tt in range(4):
                    gt = tci * 4 + tt  # global t-tile / k-tile index
                    ps = pjps.tile([128, TCH], F32, tag="pj", name=f"pjv{gt}")
                    for dc in range(DC):
                        nc.tensor.matmul(
                            ps[:, : HPC * DH],
                            xt[:, dc, tt * 128 : (tt + 1) * 128],
                            wv[:, dc, :],
                            start=(dc == 0),
                            stop=(dc == DC - 1),
                        )
                    nc.vector.tensor_copy(
                        vaug[:, gt, :, 0:DH],
                        ps[:, : HPC * DH].rearrange("p (h d) -> p h d", h=HPC),
                    )

            def attn_chunk(qc):
                qsl = slice(qc * TCH, (qc + 1) * TCH)
                nkt = 4 * qc + 4  # causal: k-tiles 0..4qc+3
                for h in range(HPC):
                    pr, par = divmod(h, 2)
                    qrh = qt[pr][par * 64 : par * 64 + 64, qsl]
                    opsum = ops.tile([128, TCH], F32, tag="o", name=f"o{qc}_{h}")
                    for kb in range(0, nkt, 2):
                        spt = sps.tile([128, 2, TCH], F32, tag="s", name=f"s{qc}_{h}")
                        pt = ptp.tile([128, 2, TCH], F32R, tag="pt", name=f"pt{qc}_{h}")
                        for j in range(2):
                            ktile = kb + j
                            nc.tensor.matmul(
                                spt[:, j, :],
                                ktt[pr][
                                    par * 64 : par * 64 + 64,
                                    ktile * 128 : (ktile + 1) * 128,
                                ],
                                qrh,
                                start=True,
                                stop=True,
                            )
                        nc.scalar.activation(
                            pt.rearrange("p a b -> p (a b)"),
                            spt.rearrange("p a b -> p (a b)"),
                            EXP,
                            bias=0.0,
                            scale=0.125,
                        )
                        for j in range(2):
                            ktile = kb + j
                            if ktile >= 4 * qc:  # diagonal region: mask k > q
                                nc.gpsimd.affine_select(
                                    out=pt[:, j, :],
                                    in_=pt[:, j, :],
                                    compare_op=mybir.AluOpType.is_ge,
                                    fill=0.0,
                                    base=512 * qc - 128 * ktile,
                                    pattern=[[1, TCH]],
                                    channel_multiplier=-1,
                                )
                        for j in range(2):
                            ktile = kb + j
                            nc.tensor.matmul(
                                opsum[0 : DH + 1, :],
                                vaug[:, ktile, h, :],
                                pt[:, j, :],
                                start=(ktile == 0),
                                stop=(ktile == nkt - 1),
                            )
                    # evac O' fast (frees the PSUM bank), normalize from SBUF
                    oraw = misp.tile([128, TCH], F32, tag="oraw", name=f"or{qc}_{h}")
                    nc.vector.tensor_copy(oraw[0 : DH + 1, :], opsum[0 : DH + 1, :])
                    rec0 = misp.tile([1, TCH], F32, tag="rec0", name=f"rc{qc}_{h}")
                    nc.gpsimd.dma_start(rec0[:], oraw[DH : DH + 1, :])
                    nc.vector.reciprocal(rec0[:], rec0[:])
                    bc = misp.tile([64, TCH], F32, tag="bc", name=f"bc{qc}_{h}")
                    nc.gpsimd.partition_broadcast(bc[:], rec0[:])
                    if par == 0:
                        nc.vector.tensor_mul(
                            ot[pr][0:64, qsl], oraw[0:64, :], bc[:]
                        )
                    else:
                        tmpo = misp.tile([64, TCH], F32R, tag="tmpo", name=f"tp{qc}_{h}")
                        nc.vector.tensor_mul(tmpo[:], oraw[0:64, :], bc[:])
                        nc.scalar.dma_start(ot[pr][64:128, qsl], tmpo[:])

                for tt in range(4):
                    gtt = qc * 4 + tt
                    for ni in range(2):
                        ypsum = yps.tile([128, TCH], F32, tag="y", name=f"y{gtt}_{ni}")
                        for p2 in range(2):
                            nc.tensor.matmul(
                                ypsum[:],
                                ot[p2][:, gtt * 128 : (gtt + 1) * 128],
                                wo[:, p2, ni * TCH : (ni + 1) * TCH],
                                start=(p2 == 0),
                                stop=(p2 == 1),
                            )
                        ysb = ysbp.tile([128, TCH], F32, tag="ysb", name=f"ys{gtt}_{ni}")
                        nc.scalar.copy(ysb[:], ypsum[:])
                        nc.sync.dma_start(
                            y_d.ap()[
                                gtt * 128 : (gtt + 1) * 128,
                                ni * TCH : (ni + 1) * TCH,
                            ],
                            ysb[:],
                        )

            # Interleaved emission: proj stays one chunk ahead of attention so
            # PE always has dense independent work during softmax chain stalls.
            proj_chunk(0)
            proj_chunk(1)
            attn_chunk(0)
            proj_chunk(2)
            attn_chunk(1)
            proj_chunk(3)
            attn_chunk(2)
            attn_chunk(3)
    nc.compile()
    return nc


_NC_CACHE = []


def _get_nc():
    if not _NC_CACHE:
        _NC_CACHE.append(_build())
    return _NC_CACHE[0]


_LAST_RESULTS = []  # stashed BassKernelResults for test harness introspection


def kernel(x, Wqkv, Wout, _trace=False, **_trace_kwargs):
    x = np.asarray(x, dtype=np.float32)
    Wqkv = np.asarray(Wqkv, dtype=np.float32)
    Wout = np.asarray(Wout, dtype=np.float32)

    nc = _get_nc()
    in_maps = []
    for c in range(NCORES):
        b, g = divmod(c, HPC)
        cols = slice(g * HPC * DH, (g + 1) * HPC * DH)
        rows = slice(g * HPC * DH, (g + 1) * HPC * DH)
        in_maps.append(
            {
                "xT": np.ascontiguousarray(x[b].T),
                "wq": np.ascontiguousarray(Wqkv[:, 0:D][:, cols]),
                "wk": np.ascontiguousarray(Wqkv[:, D : 2 * D][:, cols]),
                "wv": np.ascontiguousarray(Wqkv[:, 2 * D : 3 * D][:, cols]),
                "wo": np.ascontiguousarray(Wout[rows, :]),
                "y": None,  # outputs are allocated by the runner
            }
        )
    for m in in_maps:
        m.pop("y")

    res = run_bass_kernel_spmd(
        nc, in_maps, core_ids=list(range(NCORES)), trace=_trace, **_trace_kwargs
    )
    _LAST_RESULTS.clear()
    _LAST_RESULTS.append(res)

    out = np.zeros((B, T, D), dtype=np.float32)
    for c in range(NCORES):
        b = c // HPC
        out[b] += res.results[c]["y"]
    return out


# revision 11
# speedup vs baseline: 1.0348x; 1.0060x over previous
"""Trainium2 Bass kernel for EnhancedAttention (B=2, T=2048, D=1024, H=16, DH=64).

Sharding: 8 cores = 2 batches x 4 head-groups (4 heads each). No collectives;
each core computes a partial out-projection and the host sums the 4 partials
per batch.

Per-core dataflow (all "transposed" space, float32r matmuls):
  Q^T,K^T [dh,t] from W^T @ x^T; RoPE via strip-swap DMA + full-width DVE ops;
  V natural [t,dh] augmented with a ones column so softmax denominators fall
  out of the attention matmul; causal handled by skipping above-diagonal
  k-tiles entirely + gpsimd.affine_select on diagonal tiles; normalization
  applied to O^T (per-head) before the out-projection.
"""
import os
import sys

for _p in ("/opt/trn_rl_repo", "/root/.axon_site/_ro/trn_rl_repo"):
    if os.path.isdir(_p) and _p not in sys.path:
        sys.path.append(_p)

import numpy as np

import concourse.bass as bass  # noqa: F401
import concourse.tile as tile
from concourse import bacc, mybir
from concourse.bass_utils import run_bass_kernel_spmd

B, T, D = 2, 2048, 1024
H, DH = 16, 64
HPC = 4  # heads per core
NCORES = 8
ROPE_THETA = 10000.0

F32 = mybir.dt.float32
F32R = mybir.dt.float32r

TCH = 512  # t-chunk (q-chunk) size
TC = T // TCH  # 4
DC = D // 128  # 8 contraction chunks
NKT = T // 128  # 16 k-tiles


def _rope_tables():
    inv = 1.0 / (ROPE_THETA ** (np.arange(0, DH, 2, dtype=np.float64) / DH))
    f = np.arange(T, dtype=np.float64)[:, None] * inv[None, :]  # [T, 32]
    cos = np.cos(f).T.astype(np.float32)  # [32, T]
    sin = np.sin(f).T.astype(np.float32)
    cs1 = np.ascontiguousarray(np.tile(cos, (4, 1)))  # [128, T]
    cs2 = np.ascontiguousarray(np.concatenate([-sin, sin, -sin, sin], axis=0))
    return cs1, cs2


def _build():
    nc = bacc.Bacc("TRN2", target_bir_lowering=False, debug=False, num_devices=NCORES)
    xT_d = nc.dram_tensor("xT", [D, T], F32R, kind="ExternalInput")
    wq_d = nc.dram_tensor("wq", [D, HPC * DH], F32R, kind="ExternalInput")
    wk_d = nc.dram_tensor("wk", [D, HPC * DH], F32R, kind="ExternalInput")
    wv_d = nc.dram_tensor("wv", [D, HPC * DH], F32R, kind="ExternalInput")
    wo_d = nc.dram_tensor("wo", [HPC * DH, D], F32R, kind="ExternalInput")
    y_d = nc.dram_tensor("y", [T, D], F32, kind="ExternalOutput")

    cs1_np, cs2_np = _rope_tables()
    cs1_d = nc.inline_tensor(cs1_np, "cs1")
    cs2_d = nc.inline_tensor(cs2_np, "cs2")

    EXP = mybir.ActivationFunctionType.Exp

    import contextlib
    with tile.TileContext(nc) as tc:
        with (
            contextlib.ExitStack() as _ctx,
            tc.tile_pool(name="sb", bufs=1) as sb,
            tc.tile_pool(name="xtp", bufs=2) as xtp,
            tc.tile_pool(name="ropep", bufs=2) as ropep,
            tc.tile_pool(name="ptp", bufs=6) as ptp,
            tc.tile_pool(name="misp", bufs=2) as misp,
            tc.tile_pool(name="ysbp", bufs=2) as ysbp,
        ):
            wq = sb.tile([128, DC, HPC * DH], F32R)
            wk = sb.tile([128, DC, HPC * DH], F32R)
            wv = sb.tile([128, DC, HPC * DH], F32R)
            wo = sb.tile([128, 2, D], F32R)
            cs1 = sb.tile([128, T], F32)
            cs2 = sb.tile([128, T], F32)
            qt = [sb.tile([128, T], F32R, tag=f"qt{p}", name=f"qt{p}") for p in range(2)]
            ktt = [sb.tile([128, T], F32R, tag=f"kt{p}", name=f"kt{p}") for p in range(2)]
            vaug = sb.tile([128, NKT, HPC, DH + 1], F32R)
            ot = [sb.tile([128, T], F32R, tag=f"ot{p}", name=f"ot{p}") for p in range(2)]

            nc.scalar.dma_start(wq[:], wq_d.ap().rearrange("(c p) n -> p c n", p=128))
            nc.sync.dma_start(wk[:], wk_d.ap().rearrange("(c p) n -> p c n", p=128))
            nc.gpsimd.dma_start(wv[:], wv_d.ap().rearrange("(c p) n -> p c n", p=128))
            nc.gpsimd.dma_start(wo[:], wo_d.ap().rearrange("(c p) n -> p c n", p=128))
            nc.gpsimd.dma_start(cs1[:], cs1_d.ap())
            nc.gpsimd.dma_start(cs2[:], cs2_d.ap())
            nc.vector.memset(vaug[:, :, :, DH : DH + 1].bitcast(F32), 1.0)

            xT_r = xT_d.ap().rearrange("(c p) t -> p c t", p=128)

            # One flat PSUM pool set so proj and attention interleave freely
            pjps = _ctx.enter_context(tc.tile_pool(name="pjps", bufs=1, space="PSUM"))
            sps = _ctx.enter_context(tc.tile_pool(name="sps", bufs=2, space="PSUM"))
            ops = _ctx.enter_context(tc.tile_pool(name="ops", bufs=2, space="PSUM"))
            yps = _ctx.enter_context(tc.tile_pool(name="yps", bufs=1, space="PSUM"))

            # PE warm-up: fills the input-DMA wait so HAM reaches K=8/8
            warm = sb.tile([128, TCH], F32R, name="warm")
            nc.vector.memset(warm.bitcast(F32), 0.0)
            wps = pjps.tile([128, TCH], F32, tag="pj", name="wps")
            for wi in range(16):
                nc.tensor.matmul(
                    wps[:], warm[:, 0:128], warm[:],
                    start=(wi == 0), stop=(wi == 15),
                )

            def proj_chunk(tci):
                tsl = slice(tci * TCH, (tci + 1) * TCH)
                xt = xtp.tile([128, DC, TCH], F32R, tag="xt", name=f"xt{tci}")
                nc.sync.dma_start(xt[:, 0:4, :], xT_r[:, 0:4, tsl])
                nc.sync.dma_start(xt[:, 4:8, :], xT_r[:, 4:8, tsl])

                for w_sb, dest in ((wq, qt), (wk, ktt)):
                    for p in range(2):
                        ps = pjps.tile([128, TCH], F32, tag="pj", name=f"pj{tci}_{p}")
                        for dc in range(DC):
                            nc.tensor.matmul(
                                ps[:],
                                w_sb[:, dc, p * 128 : (p + 1) * 128],
                                xt[:, dc, :],
                                start=(dc == 0),
                                stop=(dc == DC - 1),
                            )
                        e = ropep.tile([128, TCH], F32, tag="e", name=f"e{tci}")
                        sw = ropep.tile([128, TCH], F32, tag="sw", name=f"sw{tci}")
                        nc.vector.tensor_copy(e[:], ps[:])
                        for s in range(4):
                            ring = (nc.sync, nc.scalar, nc.gpsimd)[(2 * tci + p + s) % 3]
                            ring.dma_start(
                                sw[s * 32 : (s + 1) * 32, :],
                                e[(s ^ 1) * 32 : ((s ^ 1) + 1) * 32, :],
                            )
                        nc.vector.tensor_mul(e[:], e[:], cs1[:, tsl])
                        nc.vector.tensor_mul(sw[:], sw[:], cs2[:, tsl])
                        nc.vector.tensor_add(dest[p][:, tsl], e[:], sw[:])

                for tt in range(4):
                    gt = tci * 4 + tt  # global t-tile / k-tile index
                    ps = pjps.tile([128, TCH], F32, tag="pj", name=f"pjv{gt}")
                    for dc in range(DC):
                        nc.tensor.matmul(
                            ps[:, : HPC * DH],
                            xt[:, dc, tt * 128 : (tt + 1) * 128],
                            wv[:, dc, :],
                            start=(dc == 0),
                            stop=(dc == DC - 1),
                        )
                    nc.vector.tensor_copy(
                        vaug[:, gt, :, 0:DH],
                        ps[:, : HPC * DH].rearrange("p (h d) -> p h d", h=HPC),
                    )

            def attn_chunk(qc):
                qsl = slice(qc * TCH, (qc + 1) * TCH)
                nkt = 4 * qc + 4  # causal: k-tiles 0..4qc+3
                for h in range(HPC):
                    pr, par = divmod(h, 2)
                    qrh = qt[pr][par * 64 : par * 64 + 64, qsl]
                    opsum = ops.tile([128, TCH], F32, tag="o", name=f"o{qc}_{h}")
                    for kb in range(0, nkt, 2):
                        spt = sps.tile([128, 2, TCH], F32, tag="s", name=f"s{qc}_{h}")
                        pt = ptp.tile([128, 2, TCH], F32R, tag="pt", name=f"pt{qc}_{h}")
                        for j in range(2):
                            ktile = kb + j
                            nc.tensor.matmul(
                                spt[:, j, :],
                                ktt[pr][
                                    par * 64 : par * 64 + 64,
                                    ktile * 128 : (ktile + 1) * 128,
                                ],
                                qrh,
                                start=True,
                                stop=True,
                            )
                        nc.scalar.activation(
                            pt.rearrange("p a b -> p (a b)"),
                            spt.rearrange("p a b -> p (a b)"),
                            EXP,
                            bias=0.0,
                            scale=0.125,
                        )
                        for j in range(2):
                            ktile = kb + j
                            if ktile >= 4 * qc:  # diagonal region: mask k > q
                                nc.gpsimd.affine_select(
                                    out=pt[:, j, :],
                                    in_=pt[:, j, :],
                                    compare_op=mybir.AluOpType.is_ge,
                                    fill=0.0,
                                    base=512 * qc - 128 * ktile,
                                    pattern=[[1, TCH]],
                                    channel_multiplier=-1,
                                )
                        for j in range(2):
                            ktile = kb + j
                            nc.tensor.matmul(
                                opsum[0 : DH + 1, :],
                                vaug[:, ktile, h, :],
                                pt[:, j, :],
                                start=(ktile == 0),
                                stop=(ktile == nkt - 1),
                            )
                    # evac O' fast (frees the PSUM bank), normalize from SBUF
                    oraw = misp.tile([128, TCH], F32, tag="oraw", name=f"or{qc}_{h}")
                    nc.vector.tensor_copy(oraw[0 : DH + 1, :], opsum[0 : DH + 1, :])
                    rec0 = misp.tile([1, TCH], F32, tag="rec0", name=f"rc{qc}_{h}")
                    nc.gpsimd.dma_start(rec0[:], oraw[DH : DH + 1, :])
                    nc.vector.reciprocal(rec0[:], rec0[:])
                    bc = misp.tile([64, TCH], F32, tag="bc", name=f"bc{qc}_{h}")
                    nc.gpsimd.partition_broadcast(bc[:], rec0[:])
                    if par == 0:
                        nc.vector.tensor_mul(
                            ot[pr][0:64, qsl], oraw[0:64, :], bc[:]
                        )
                    else:
                        tmpo = misp.tile([64, TCH], F32R, tag="tmpo", name=f"tp{qc}_{h}")
                        nc.vector.tensor_mul(tmpo[:], oraw[0:64, :], bc[:])
                        nc.scalar.dma_start(ot[pr][64:128, qsl], tmpo[:])

                for tt in range(4):
                    gtt = qc * 4 + tt
                    for ni in range(2):
                        ypsum = yps.tile([128, TCH], F32, tag="y", name=f"y{gtt}_{ni}")
                        for p2 in range(2):
                            nc.tensor.matmul(
                                ypsum[:],
                                ot[p2][:, gtt * 128 : (gtt + 1) * 128],
                                wo[:, p2, ni * TCH : (ni + 1) * TCH],
                                start=(p2 == 0),
                                stop=(p2 == 1),
                            )
                        ysb = ysbp.tile([128, TCH], F32, tag="ysb", name=f"ys{gtt}_{ni}")
                        nc.scalar.copy(ysb[:], ypsum[:])
                        nc.sync.dma_start(
                            y_d.ap()[
                                gtt * 128 : (gtt + 1) * 128,
                                ni * TCH : (ni + 1) * TCH,
                            ],
                            ysb[:],
                        )

            # Interleaved emission: proj stays one chunk ahead of attention so
            # PE always has dense independent work during softmax chain stalls.
            proj_chunk(0)
            proj_chunk(1)
            attn_chunk(0)
            proj_chunk(2)
            attn_chunk(1)
            proj_chunk(3)
            attn_chunk(2)
            attn_chunk(3)
    nc.compile()
    return nc


_NC_CACHE = []


def _get_nc():
    if not _NC_CACHE:
        _NC_CACHE.append(_build())
    return _NC_CACHE[0]


_LAST_RESULTS = []  # stashed BassKernelResults for test harness introspection


def kernel(x, Wqkv, Wout, _trace=False, **_trace_kwargs):
    x = np.asarray(x, dtype=np.float32)
    Wqkv = np.asarray(Wqkv, dtype=np.float32)
    Wout = np.asarray(Wout, dtype=np.float32)

    nc = _get_nc()
    in_maps = []
    for c in range(NCORES):
        b, g = divmod(c, HPC)
        cols = slice(g * HPC * DH, (g + 1) * HPC * DH)
        rows = slice(g * HPC * DH, (g + 1) * HPC * DH)
        in_maps.append(
            {
                "xT": np.ascontiguousarray(x[b].T),
                "wq": np.ascontiguousarray(Wqkv[:, 0:D][:, cols]),
                "wk": np.ascontiguousarray(Wqkv[:, D : 2 * D][:, cols]),
                "wv": np.ascontiguousarray(Wqkv[:, 2 * D : 3 * D][:, cols]),
                "wo": np.ascontiguousarray(Wout[rows, :]),
                "y": None,  # outputs are allocated by the runner
            }
        )
    for m in in_maps:
        m.pop("y")

    res = run_bass_kernel_spmd(
        nc, in_maps, core_ids=list(range(NCORES)), trace=_trace, **_trace_kwargs
    )
    _LAST_RESULTS.clear()
    _LAST_RESULTS.append(res)

    out = np.zeros((B, T, D), dtype=np.float32)
    for c in range(NCORES):
        b = c // HPC
        out[b] += res.results[c]["y"]
    return out
